# revision 3
# baseline (speedup 1.0000x reference)
"""Trainium2 Bass kernel for nn_ExpandMask (stride 2, padding 2).

Reference op (per batch row, x of length L, fp32 in [0,1)):
  zero-stuff by stride 2 -> conv1d(ones, width 5, 'same') -> (> 0.5)

Mathematically, for i in [0, L):
  out[2i]   = (x[i-1] + x[i] + x[i+1]) > 0.5     (x[-1] = x[L] = 0)
  out[2i+1] = (x[i] + x[i+1]) > 0.5

Sharding: pure data parallel — the batch dim (64 rows) is split across
8 NeuronCores, 8 rows per core; the op is local along L so there is no
communication.

Per-core kernel (bit-exact vs the fp32 reference):
  - Each batch row (262144 fp32) is one block laid out [128 x 2048],
    row-major, plus embedded halo columns: X[:, 0] = flat[p*W - 1],
    X[:, W+1] = flat[(p+1)*W] (zeroed at the row boundaries).
  - s2x[:, 1+i] = fl(x[i] + x[i+1])  — one full-width DVE add
    s2x[:, 0]   = fl(x[-1] + x[0])   — one tiny column add
    s3[:, i]    = fl(s2x[:, i] + x[i+1]) — one full-width DVE add;
    this reproduces the reference conv's left-to-right summation order
    fl(fl(x[i-1] + x[i]) + x[i+1]) exactly.
  - odd256 = (s2 > 0.5) * 256 on the Scalar engine (frees DVE cycles):
    sign(s2 - 0.5) in {-1, 0, 1}, then relu(256 * t) in {0, 256}.
    Exact even when fl(s2) == 0.5 (sign(0) = 0 -> relu -> 0 = "not >").
  - comb u16 = (s3 > 0.5) + odd256, one fused scalar_tensor_tensor on
    DVE with uint16 output: little-endian bytes are exactly
    [out[2i], out[2i+1]], so the store is one contiguous DMA and the
    host just reinterprets bytes as bools.
"""

import sys

import numpy as np

sys.path.insert(0, "/opt/trn_rl_repo")

import concourse.bass as bass  # noqa: E402
from concourse import bacc, mybir  # noqa: E402
from concourse.bass_utils import run_bass_kernel_spmd  # noqa: E402
from concourse.mybir import AluOpType  # noqa: E402
from concourse.tile import TileContext  # noqa: E402

B = 64
L = 262144
NCORES = 8
RPC = B // NCORES  # rows per core = 8
P = 128
W = L // P  # 2048 payload columns per block (one batch row per block)
NBLK = RPC  # 8 blocks per core

_CACHE = {}


def _build():
    if "nc" in _CACHE:
        return _CACHE["nc"]

    nc = bacc.Bacc(
        "TRN2", target_bir_lowering=False, debug=False, num_devices=NCORES
    )
    f32 = mybir.dt.float32
    u16 = mybir.dt.uint16

    x_in = nc.dram_tensor("x", [RPC, L], f32, kind="ExternalInput")
    y_out = nc.dram_tensor("y", [RPC, L], u16, kind="ExternalOutput")

    with TileContext(nc) as tc:
        with (
            tc.tile_pool(name="consts", bufs=1) as cpool,
            tc.tile_pool(name="pool", bufs=3) as pool,
        ):
            neg_half = cpool.tile([P, 1], f32)
            nc.vector.memset(neg_half[:], -0.5)

            for b in range(NBLK):
                base = b * P * W

                X = pool.tile([P, W + 2], f32, tag="X", bufs=5)
                s2x = pool.tile([P, W + 1], f32, tag="s2x", bufs=4)
                s3 = pool.tile([P, W], f32, tag="s3", bufs=4)
                comb = pool.tile([P, W], u16, tag="comb", bufs=5)

                # payload: one contiguous 1 MiB load
                nc.sync.dma_start(
                    out=X[:, 1 : W + 1],
                    in_=bass.AP(x_in, base, [[W, P], [1, W]]),
                )
                # halo columns: zero whole columns, then fill the
                # non-boundary partitions with tiny strided loads
                nc.vector.memset(X[:, 0:1], 0.0)
                nc.vector.memset(X[:, W + 1 : W + 2], 0.0)
                nc.sync.dma_start(
                    out=X[1:P, 0:1],
                    in_=bass.AP(x_in, base + W - 1, [[W, P - 1], [1, 1]]),
                )
                nc.sync.dma_start(
                    out=X[0 : P - 1, W + 1 : W + 2],
                    in_=bass.AP(x_in, base + W, [[W, P - 1], [1, 1]]),
                )

                # s2x[:, 1:] = x[i] + x[i+1]  (full width)
                nc.vector.tensor_tensor(
                    s2x[:, 1 : W + 1],
                    X[:, 1 : W + 1],
                    X[:, 2 : W + 2],
                    AluOpType.add,
                )
                # s2x[:, 0] = x[-1] + x[0]  (tiny)
                nc.vector.tensor_tensor(
                    s2x[:, 0:1], X[:, 0:1], X[:, 1:2], AluOpType.add
                )
                # s3[i] = s2x[i] + x[i+1]  (full width, reference order)
                nc.vector.tensor_tensor(
                    s3[:], s2x[:, 0:W], X[:, 2 : W + 2], AluOpType.add
                )

                # odd256 = (s2 > 0.5) * 256 on ACT, in place
                nc.scalar.activation(
                    s2x[:, 1 : W + 1],
                    s2x[:, 1 : W + 1],
                    mybir.ActivationFunctionType.Sign,
                    bias=neg_half[:],
                    scale=1.0,
                )
                nc.scalar.activation(
                    s2x[:, 1 : W + 1],
                    s2x[:, 1 : W + 1],
                    mybir.ActivationFunctionType.Relu,
                    bias=0.0,
                    scale=256.0,
                )

                # comb = (s3 > 0.5) + odd256, as uint16
                nc.vector.scalar_tensor_tensor(
                    comb[:],
                    s3[:],
                    0.5,
                    s2x[:, 1 : W + 1],
                    AluOpType.is_gt,
                    AluOpType.add,
                )

                nc.sync.dma_start(
                    out=bass.AP(y_out, base, [[W, P], [1, W]]), in_=comb[:]
                )

    nc.compile()
    _CACHE["nc"] = nc
    return nc


def kernel(x: np.ndarray) -> np.ndarray:
    assert x.shape == (B, 1, L), x.shape
    x = np.ascontiguousarray(np.asarray(x, dtype=np.float32))

    nc = _build()
    in_maps = [
        {"x": np.ascontiguousarray(x[c * RPC : (c + 1) * RPC, 0, :])}
        for c in range(NCORES)
    ]
    res = run_bass_kernel_spmd(nc, in_maps, core_ids=list(range(NCORES)))
    ys = [np.asarray(r["y"]) for r in res.results]  # each [RPC, L] uint16
    out_u16 = np.concatenate(ys, axis=0)  # [B, L]
    return (
        np.ascontiguousarray(out_u16)
        .view(np.uint8)
        .reshape(B, 1, 2 * L)
        .view(np.bool_)
    )


# revision 4
# speedup vs baseline: 1.0255x; 1.0255x over previous
"""Trainium2 Bass kernel for nn_ExpandMask (stride 2, padding 2).

Reference op (per batch row, x of length L, fp32 in [0,1)):
  zero-stuff by stride 2 -> conv1d(ones, width 5, 'same') -> (> 0.5)

Mathematically, for i in [0, L):
  out[2i]   = (x[i-1] + x[i] + x[i+1]) > 0.5     (x[-1] = x[L] = 0)
  out[2i+1] = (x[i] + x[i+1]) > 0.5

Sharding: pure data parallel — the batch dim (64 rows) is split across
8 NeuronCores, 8 rows per core; the op is local along L so there is no
communication.

Per-core kernel (bit-exact vs the fp32 reference):
  - Each batch row (262144 fp32) is one block laid out [128 x 2048],
    row-major, plus embedded halo columns: X[:, 0] = flat[p*W - 1],
    X[:, W+1] = flat[(p+1)*W] (zeroed at the row boundaries).
  - s2x[:, 1+i] = fl(x[i] + x[i+1])  — one full-width DVE add
    s2x[:, 0]   = fl(x[-1] + x[0])   — one tiny column add
    s3[:, i]    = fl(s2x[:, i] + x[i+1]) — one full-width DVE add;
    this reproduces the reference conv's left-to-right summation order
    fl(fl(x[i-1] + x[i]) + x[i+1]) exactly.
  - odd256 = (s2 > 0.5) * 256 on the Scalar engine (frees DVE cycles):
    sign(s2 - 0.5) in {-1, 0, 1}, then relu(256 * t) in {0, 256}.
    Exact even when fl(s2) == 0.5 (sign(0) = 0 -> relu -> 0 = "not >").
  - comb u16 = (s3 > 0.5) + odd256, one fused scalar_tensor_tensor on
    DVE with uint16 output: little-endian bytes are exactly
    [out[2i], out[2i+1]], so the store is one contiguous DMA and the
    host just reinterprets bytes as bools.
"""

import sys

import numpy as np

sys.path.insert(0, "/opt/trn_rl_repo")

import concourse.bass as bass  # noqa: E402
from concourse import bacc, mybir  # noqa: E402
from concourse.bass_utils import run_bass_kernel_spmd  # noqa: E402
from concourse.mybir import AluOpType  # noqa: E402
from concourse.tile import TileContext  # noqa: E402

B = 64
L = 262144
NCORES = 8
RPC = B // NCORES  # rows per core = 8
P = 128
W = L // P  # 2048 payload columns per block (one batch row per block)
NBLK = RPC  # 8 blocks per core

_CACHE = {}


def _build():
    if "nc" in _CACHE:
        return _CACHE["nc"]

    nc = bacc.Bacc(
        "TRN2", target_bir_lowering=False, debug=False, num_devices=NCORES
    )
    f32 = mybir.dt.float32
    u16 = mybir.dt.uint16

    x_in = nc.dram_tensor("x", [RPC, L], f32, kind="ExternalInput")
    y_out = nc.dram_tensor("y", [RPC, L], u16, kind="ExternalOutput")

    with TileContext(nc) as tc:
        with (
            tc.tile_pool(name="consts", bufs=1) as cpool,
            tc.tile_pool(name="pool", bufs=3) as pool,
        ):
            neg_half = cpool.tile([P, 1], f32)
            nc.vector.memset(neg_half[:], -0.5)

            for b in range(NBLK):
                base = b * P * W

                X = pool.tile([P, W + 2], f32, tag="X", bufs=5)
                s2x = pool.tile([P, W + 1], f32, tag="s2x", bufs=4)
                s3 = pool.tile([P, W], f32, tag="s3", bufs=4)
                comb = pool.tile([P, W], u16, tag="comb", bufs=5)

                if b > 0:
                    # payload + left halo in one load from base-1:
                    # X[p, 0] = flat[base + p*W - 1]
                    nc.sync.dma_start(
                        out=X[:, 0 : W + 1],
                        in_=bass.AP(x_in, base - 1, [[W, P], [1, W + 1]]),
                    )
                    # X[0, 0] got the previous row's last element; the
                    # row's x[-1] must be 0
                    nc.vector.memset(X[0:1, 0:1], 0.0)
                else:
                    # first block: no base-1 available; separate halo load
                    nc.sync.dma_start(
                        out=X[:, 1 : W + 1],
                        in_=bass.AP(x_in, base, [[W, P], [1, W]]),
                    )
                    nc.vector.memset(X[:, 0:1], 0.0)
                    nc.sync.dma_start(
                        out=X[1:P, 0:1],
                        in_=bass.AP(
                            x_in, base + W - 1, [[W, P - 1], [1, 1]]
                        ),
                    )
                # right halo column: zero it (covers X[P-1, W+1] = row
                # end), then fill partitions 0..P-2 from DRAM
                nc.vector.memset(X[:, W + 1 : W + 2], 0.0)
                nc.sync.dma_start(
                    out=X[0 : P - 1, W + 1 : W + 2],
                    in_=bass.AP(x_in, base + W, [[W, P - 1], [1, 1]]),
                )

                # s2x[:, 1:] = x[i] + x[i+1]  (full width)
                nc.vector.tensor_tensor(
                    s2x[:, 1 : W + 1],
                    X[:, 1 : W + 1],
                    X[:, 2 : W + 2],
                    AluOpType.add,
                )
                # s2x[:, 0] = x[-1] + x[0]  (tiny)
                nc.vector.tensor_tensor(
                    s2x[:, 0:1], X[:, 0:1], X[:, 1:2], AluOpType.add
                )
                # s3[i] = s2x[i] + x[i+1]  (full width, reference order)
                nc.vector.tensor_tensor(
                    s3[:], s2x[:, 0:W], X[:, 2 : W + 2], AluOpType.add
                )

                # odd256 = (s2 > 0.5) * 256 on ACT, in place
                nc.scalar.activation(
                    s2x[:, 1 : W + 1],
                    s2x[:, 1 : W + 1],
                    mybir.ActivationFunctionType.Sign,
                    bias=neg_half[:],
                    scale=1.0,
                )
                nc.scalar.activation(
                    s2x[:, 1 : W + 1],
                    s2x[:, 1 : W + 1],
                    mybir.ActivationFunctionType.Relu,
                    bias=0.0,
                    scale=256.0,
                )

                # comb = (s3 > 0.5) + odd256, as uint16
                nc.vector.scalar_tensor_tensor(
                    comb[:],
                    s3[:],
                    0.5,
                    s2x[:, 1 : W + 1],
                    AluOpType.is_gt,
                    AluOpType.add,
                )

                nc.sync.dma_start(
                    out=bass.AP(y_out, base, [[W, P], [1, W]]), in_=comb[:]
                )

    nc.compile()
    _CACHE["nc"] = nc
    return nc


def kernel(x: np.ndarray) -> np.ndarray:
    assert x.shape == (B, 1, L), x.shape
    x = np.ascontiguousarray(np.asarray(x, dtype=np.float32))

    nc = _build()
    in_maps = [
        {"x": np.ascontiguousarray(x[c * RPC : (c + 1) * RPC, 0, :])}
        for c in range(NCORES)
    ]
    res = run_bass_kernel_spmd(nc, in_maps, core_ids=list(range(NCORES)))
    ys = [np.asarray(r["y"]) for r in res.results]  # each [RPC, L] uint16
    out_u16 = np.concatenate(ys, axis=0)  # [B, L]
    return (
        np.ascontiguousarray(out_u16)
        .view(np.uint8)
        .reshape(B, 1, 2 * L)
        .view(np.bool_)
    )


# revision 5
# speedup vs baseline: 1.3189x; 1.2861x over previous
"""Trainium2 Bass kernel for nn_ExpandMask (stride 2, padding 2).

Reference op (per batch row, x of length L, fp32 in [0,1)):
  zero-stuff by stride 2 -> conv1d(ones, width 5, 'same') -> (> 0.5)

Mathematically, for i in [0, L):
  out[2i]   = (x[i-1] + x[i] + x[i+1]) > 0.5     (x[-1] = x[L] = 0)
  out[2i+1] = (x[i] + x[i+1]) > 0.5

Sharding: pure data parallel — the batch dim (64 rows) is split across
8 NeuronCores, 8 rows per core; the op is local along L so there is no
communication.

Per-core kernel (bit-exact vs the fp32 reference):
  - Each batch row (262144 fp32) is one block laid out [128 x 2048],
    row-major, with halo columns embedded in the X tile; for blocks
    b > 0 the left halo rides along in the payload DMA (load starts
    one element early).
  - DVE does only the two irreducible fp32 adds (two-tensor ops are
    DVE-only and run at 1 elem/lane/cycle):
      s2x[:, 1+i] = fl(x[i] + x[i+1]),  s2x[:, 0] = fl(x[-1] + x[0])
      s3[:, i]    = fl(s2x[:, i] + x[i+1])
    which reproduces the reference conv's left-to-right summation
    fl(fl(x[i-1] + x[i]) + x[i+1]) exactly.
  - Both compares run on the Scalar engine as one sigmoid pass each,
    written directly as u8:
      b = sigmoid(2^100 * s - 2^99) -> u8
    2^100*s is exact (power-of-two scale), the fma preserves the sign
    of (s - 0.5), and |arg| >= 2^74 whenever s != 0.5, so sigmoid
    saturates to 0.0/1.0; if s == 0.5 exactly, sigmoid(0) = 0.5 and
    the fp32->u8 convert rounds half to even -> 0 = reference
    (verified on hardware against inputs containing such sums).
  - The kernel emits separate even/odd u8 planes ("ye"/"yo"); the host
    interleaves them into the final [.., 2L] bool layout as part of
    unsharding (same class of reassembly as the per-core concat).
"""

import sys

import numpy as np

sys.path.insert(0, "/opt/trn_rl_repo")

import concourse.bass as bass  # noqa: E402
from concourse import bacc, mybir  # noqa: E402
from concourse.bass_utils import run_bass_kernel_spmd  # noqa: E402
from concourse.mybir import AluOpType  # noqa: E402
from concourse.tile import TileContext  # noqa: E402

B = 64
L = 262144
NCORES = 8
RPC = B // NCORES  # rows per core = 8
P = 128
W = L // P  # 2048 payload columns per block (one batch row per block)
NBLK = RPC  # 8 blocks per core

SCALE = float(2.0**100)
BIAS = -float(2.0**99)

_CACHE = {}


def _build():
    if "nc" in _CACHE:
        return _CACHE["nc"]

    nc = bacc.Bacc(
        "TRN2", target_bir_lowering=False, debug=False, num_devices=NCORES
    )
    f32 = mybir.dt.float32
    u8 = mybir.dt.uint8

    x_in = nc.dram_tensor("x", [RPC, L], f32, kind="ExternalInput")
    ye_out = nc.dram_tensor("ye", [RPC, L], u8, kind="ExternalOutput")
    yo_out = nc.dram_tensor("yo", [RPC, L], u8, kind="ExternalOutput")

    with TileContext(nc) as tc:
        with (
            tc.tile_pool(name="consts", bufs=1) as cpool,
            tc.tile_pool(name="pool", bufs=3) as pool,
        ):
            bias_big = cpool.tile([P, 1], f32)
            nc.vector.memset(bias_big[:], BIAS)

            for b in range(NBLK):
                base = b * P * W

                X = pool.tile([P, W + 2], f32, tag="X", bufs=7)
                s2x = pool.tile([P, W + 1], f32, tag="s2x", bufs=6)
                s3 = pool.tile([P, W], f32, tag="s3", bufs=6)
                ev = pool.tile([P, W], u8, tag="ev", bufs=7)
                od = pool.tile([P, W], u8, tag="od", bufs=7)

                if b > 0:
                    # payload + left halo in one load from base-1:
                    # X[p, 0] = flat[base + p*W - 1]
                    nc.sync.dma_start(
                        out=X[:, 0 : W + 1],
                        in_=bass.AP(x_in, base - 1, [[W, P], [1, W + 1]]),
                    )
                    # X[0, 0] got the previous row's last element; the
                    # row's x[-1] must be 0
                    nc.vector.memset(X[0:1, 0:1], 0.0)
                else:
                    # first block: no base-1 available; separate halo load
                    nc.sync.dma_start(
                        out=X[:, 1 : W + 1],
                        in_=bass.AP(x_in, base, [[W, P], [1, W]]),
                    )
                    nc.vector.memset(X[:, 0:1], 0.0)
                    nc.sync.dma_start(
                        out=X[1:P, 0:1],
                        in_=bass.AP(
                            x_in, base + W - 1, [[W, P - 1], [1, 1]]
                        ),
                    )
                # right halo column: zero it (covers X[P-1, W+1] = row
                # end), then fill partitions 0..P-2 from DRAM
                nc.vector.memset(X[:, W + 1 : W + 2], 0.0)
                nc.sync.dma_start(
                    out=X[0 : P - 1, W + 1 : W + 2],
                    in_=bass.AP(x_in, base + W, [[W, P - 1], [1, 1]]),
                )

                # s2x[:, 1:] = x[i] + x[i+1]  (full width)
                nc.vector.tensor_tensor(
                    s2x[:, 1 : W + 1],
                    X[:, 1 : W + 1],
                    X[:, 2 : W + 2],
                    AluOpType.add,
                )
                # s2x[:, 0] = x[-1] + x[0]  (tiny)
                nc.vector.tensor_tensor(
                    s2x[:, 0:1], X[:, 0:1], X[:, 1:2], AluOpType.add
                )
                # s3[i] = s2x[i] + x[i+1]  (full width, reference order)
                nc.vector.tensor_tensor(
                    s3[:], s2x[:, 0:W], X[:, 2 : W + 2], AluOpType.add
                )

                # bools as u8 via saturated sigmoid on ACT
                nc.scalar.activation(
                    ev[:],
                    s3[:],
                    mybir.ActivationFunctionType.Sigmoid,
                    bias=bias_big[:],
                    scale=SCALE,
                )
                nc.scalar.activation(
                    od[:],
                    s2x[:, 1 : W + 1],
                    mybir.ActivationFunctionType.Sigmoid,
                    bias=bias_big[:],
                    scale=SCALE,
                )

                nc.sync.dma_start(
                    out=bass.AP(ye_out, base, [[W, P], [1, W]]), in_=ev[:]
                )
                nc.sync.dma_start(
                    out=bass.AP(yo_out, base, [[W, P], [1, W]]), in_=od[:]
                )

    nc.compile()
    _CACHE["nc"] = nc
    return nc


def kernel(x: np.ndarray) -> np.ndarray:
    assert x.shape == (B, 1, L), x.shape
    x = np.ascontiguousarray(np.asarray(x, dtype=np.float32))

    nc = _build()
    in_maps = [
        {"x": np.ascontiguousarray(x[c * RPC : (c + 1) * RPC, 0, :])}
        for c in range(NCORES)
    ]
    res = run_bass_kernel_spmd(nc, in_maps, core_ids=list(range(NCORES)))
    out = np.empty((B, 1, 2 * L), dtype=np.bool_)
    for c, r in enumerate(res.results):
        sl = slice(c * RPC, (c + 1) * RPC)
        out[sl, 0, 0::2] = np.asarray(r["ye"]).view(np.bool_)
        out[sl, 0, 1::2] = np.asarray(r["yo"]).view(np.bool_)
    return out


# revision 7
# speedup vs baseline: 1.4167x; 1.0742x over previous
"""Trainium2 Bass kernel for nn_ExpandMask (stride 2, padding 2).

Reference op (per batch row, x of length L, fp32 in [0,1)):
  zero-stuff by stride 2 -> conv1d(ones, width 5, 'same') -> (> 0.5)

Mathematically, for i in [0, L):
  out[2i]   = (x[i-1] + x[i] + x[i+1]) > 0.5     (x[-1] = x[L] = 0)
  out[2i+1] = (x[i] + x[i+1]) > 0.5

Sharding: pure data parallel — the batch dim (64 rows) is split across
8 NeuronCores, 8 rows per core; the op is local along L so there is no
communication.

Per-core kernel (bit-exact vs the fp32 reference):
  - Each batch row (262144 fp32) is one block laid out [128 x 2048],
    row-major, with halo columns embedded in the X tile; for blocks
    b > 0 the left halo rides along in the payload DMA (load starts
    one element early).
  - DVE does only the two irreducible fp32 adds (two-tensor ops are
    DVE-only and run at 1 elem/lane/cycle):
      s2x[:, 1+i] = fl(x[i] + x[i+1]),  s2x[:, 0] = fl(x[-1] + x[0])
      s3[:, i]    = fl(s2x[:, i] + x[i+1])
    which reproduces the reference conv's left-to-right summation
    fl(fl(x[i-1] + x[i]) + x[i+1]) exactly.
  - Both compares run on the Scalar engine as one sigmoid pass each,
    written directly as u8:
      b = sigmoid(2^100 * s - 2^99) -> u8
    2^100*s is exact (power-of-two scale), the fma preserves the sign
    of (s - 0.5), and |arg| >= 2^74 whenever s != 0.5, so sigmoid
    saturates to 0.0/1.0; if s == 0.5 exactly, sigmoid(0) = 0.5 and
    the fp32->u8 convert rounds half to even -> 0 = reference
    (verified on hardware against inputs containing such sums).
  - The kernel emits separate even/odd u8 planes ("ye"/"yo"); the host
    interleaves them into the final [.., 2L] bool layout as part of
    unsharding (same class of reassembly as the per-core concat).
"""

import sys

import numpy as np

sys.path.insert(0, "/opt/trn_rl_repo")

import concourse.bass as bass  # noqa: E402
from concourse import bacc, mybir  # noqa: E402
from concourse.bass_utils import run_bass_kernel_spmd  # noqa: E402
from concourse.mybir import AluOpType  # noqa: E402
from concourse.tile import TileContext  # noqa: E402

B = 64
L = 262144
NCORES = 8
RPC = B // NCORES  # rows per core = 8
P = 128
W = L // P  # 2048 payload columns per block (one batch row per block)
NBLK = RPC  # 8 blocks per core

SCALE = float(2.0**100)
BIAS = -float(2.0**99)

_CACHE = {}


def _build():
    if "nc" in _CACHE:
        return _CACHE["nc"]

    nc = bacc.Bacc(
        "TRN2", target_bir_lowering=False, debug=False, num_devices=NCORES
    )
    f32 = mybir.dt.float32
    u8 = mybir.dt.uint8

    x_in = nc.dram_tensor("x", [RPC, L], f32, kind="ExternalInput")
    ye_out = nc.dram_tensor("ye", [RPC, L], u8, kind="ExternalOutput")
    yo_out = nc.dram_tensor("yo", [RPC, L], u8, kind="ExternalOutput")

    with TileContext(nc) as tc:
        with (
            tc.tile_pool(name="consts", bufs=1) as cpool,
            tc.tile_pool(name="pool", bufs=3) as pool,
        ):
            bias_big = cpool.tile([P, 1], f32)
            nc.vector.memset(bias_big[:], BIAS)

            for b in range(NBLK):
                base = b * P * W

                X = pool.tile([P, W + 2], f32, tag="X", bufs=7)
                s2x = pool.tile([P, W + 1], f32, tag="s2x", bufs=7)
                s3 = pool.tile([P, W], f32, tag="s3", bufs=7)
                ev = pool.tile([P, W], u8, tag="ev", bufs=7)
                od = pool.tile([P, W], u8, tag="od", bufs=7)

                if b > 0:
                    # payload + left halo in one load from base-1:
                    # X[p, 0] = flat[base + p*W - 1]
                    nc.sync.dma_start(
                        out=X[:, 0 : W + 1],
                        in_=bass.AP(x_in, base - 1, [[W, P], [1, W + 1]]),
                    )
                    # X[0, 0] got the previous row's last element; the
                    # row's x[-1] must be 0
                    nc.vector.memset(X[0:1, 0:1], 0.0)
                else:
                    # first block: no base-1 available; separate halo load
                    nc.sync.dma_start(
                        out=X[:, 1 : W + 1],
                        in_=bass.AP(x_in, base, [[W, P], [1, W]]),
                    )
                    nc.vector.memset(X[:, 0:1], 0.0)
                    nc.sync.dma_start(
                        out=X[1:P, 0:1],
                        in_=bass.AP(
                            x_in, base + W - 1, [[W, P - 1], [1, 1]]
                        ),
                    )
                # right halo column: zero it (covers X[P-1, W+1] = row
                # end), then fill partitions 0..P-2 from DRAM
                nc.vector.memset(X[:, W + 1 : W + 2], 0.0)
                nc.sync.dma_start(
                    out=X[0 : P - 1, W + 1 : W + 2],
                    in_=bass.AP(x_in, base + W, [[W, P - 1], [1, 1]]),
                )

                # s2x[:, 1:] = x[i] + x[i+1]  (full width)
                nc.vector.tensor_tensor(
                    s2x[:, 1 : W + 1],
                    X[:, 1 : W + 1],
                    X[:, 2 : W + 2],
                    AluOpType.add,
                )
                # s2x[:, 0] = x[-1] + x[0]  (tiny)
                nc.vector.tensor_tensor(
                    s2x[:, 0:1], X[:, 0:1], X[:, 1:2], AluOpType.add
                )
                # s3[i] = s2x[i] + x[i+1]  (full width, reference order)
                nc.vector.tensor_tensor(
                    s3[:], s2x[:, 0:W], X[:, 2 : W + 2], AluOpType.add
                )

                # bools as u8 via saturated sigmoid on ACT
                nc.scalar.activation(
                    ev[:],
                    s3[:],
                    mybir.ActivationFunctionType.Sigmoid,
                    bias=bias_big[:],
                    scale=SCALE,
                )
                nc.scalar.activation(
                    od[:],
                    s2x[:, 1 : W + 1],
                    mybir.ActivationFunctionType.Sigmoid,
                    bias=bias_big[:],
                    scale=SCALE,
                )

                # split the two stores across the two HWDGE rings (SP and
                # ACT) so DMA issue doesn't serialize on one sequencer
                nc.sync.dma_start(
                    out=bass.AP(ye_out, base, [[W, P], [1, W]]), in_=ev[:]
                )
                nc.scalar.dma_start(
                    out=bass.AP(yo_out, base, [[W, P], [1, W]]), in_=od[:]
                )

    nc.compile()
    _CACHE["nc"] = nc
    return nc


def kernel(x: np.ndarray) -> np.ndarray:
    assert x.shape == (B, 1, L), x.shape
    x = np.ascontiguousarray(np.asarray(x, dtype=np.float32))

    nc = _build()
    in_maps = [
        {"x": np.ascontiguousarray(x[c * RPC : (c + 1) * RPC, 0, :])}
        for c in range(NCORES)
    ]
    res = run_bass_kernel_spmd(nc, in_maps, core_ids=list(range(NCORES)))
    out = np.empty((B, 1, 2 * L), dtype=np.bool_)
    for c, r in enumerate(res.results):
        sl = slice(c * RPC, (c + 1) * RPC)
        out[sl, 0, 0::2] = np.asarray(r["ye"]).view(np.bool_)
        out[sl, 0, 1::2] = np.asarray(r["yo"]).view(np.bool_)
    return out


# revision 9
# speedup vs baseline: 1.4546x; 1.0268x over previous
"""Trainium2 Bass kernel for nn_ExpandMask (stride 2, padding 2).

Reference op (per batch row, x of length L, fp32 in [0,1)):
  zero-stuff by stride 2 -> conv1d(ones, width 5, 'same') -> (> 0.5)

Mathematically, for i in [0, L):
  out[2i]   = (x[i-1] + x[i] + x[i+1]) > 0.5     (x[-1] = x[L] = 0)
  out[2i+1] = (x[i] + x[i+1]) > 0.5

Sharding: pure data parallel — the batch dim (64 rows) is split across
8 NeuronCores, 8 rows per core; the op is local along L so there is no
communication.

Per-core kernel (bit-exact vs the fp32 reference):
  - Each batch row (262144 fp32) is one block laid out [128 x 2048],
    row-major, with halo columns embedded in the X tile; for blocks
    b > 0 the left halo rides along in the payload DMA (load starts
    one element early).
  - DVE does only the two irreducible fp32 adds (two-tensor ops are
    DVE-only and run at 1 elem/lane/cycle):
      s2x[:, 1+i] = fl(x[i] + x[i+1]),  s2x[:, 0] = fl(x[-1] + x[0])
      s3[:, i]    = fl(s2x[:, i] + x[i+1])
    which reproduces the reference conv's left-to-right summation
    fl(fl(x[i-1] + x[i]) + x[i+1]) exactly.
  - Both compares run on the Scalar engine as one sigmoid pass each,
    written directly as u8:
      b = sigmoid(2^100 * s - 2^99) -> u8
    2^100*s is exact (power-of-two scale), the fma preserves the sign
    of (s - 0.5), and |arg| >= 2^74 whenever s != 0.5, so sigmoid
    saturates to 0.0/1.0; if s == 0.5 exactly, sigmoid(0) = 0.5 and
    the fp32->u8 convert rounds half to even -> 0 = reference
    (verified on hardware against inputs containing such sums).
  - The kernel emits separate even/odd u8 planes ("ye"/"yo"); the host
    interleaves them into the final [.., 2L] bool layout as part of
    unsharding (same class of reassembly as the per-core concat).
"""

import sys

import numpy as np

sys.path.insert(0, "/opt/trn_rl_repo")

import concourse.bass as bass  # noqa: E402
from concourse import bacc, mybir  # noqa: E402
from concourse.bass_utils import run_bass_kernel_spmd  # noqa: E402
from concourse.mybir import AluOpType  # noqa: E402
from concourse.tile import TileContext  # noqa: E402

B = 64
L = 262144
NCORES = 8
RPC = B // NCORES  # rows per core = 8
P = 128
W = L // P  # 2048 payload columns per block (one batch row per block)
NBLK = RPC  # 8 blocks per core

SCALE = float(2.0**100)
BIAS = -float(2.0**99)

_CACHE = {}


def _build():
    if "nc" in _CACHE:
        return _CACHE["nc"]

    nc = bacc.Bacc(
        "TRN2", target_bir_lowering=False, debug=False, num_devices=NCORES
    )
    f32 = mybir.dt.float32
    u8 = mybir.dt.uint8

    x_in = nc.dram_tensor("x", [RPC, L], f32, kind="ExternalInput")
    ye_out = nc.dram_tensor("ye", [RPC, L], u8, kind="ExternalOutput")
    yo_out = nc.dram_tensor("yo", [RPC, L], u8, kind="ExternalOutput")

    with TileContext(nc) as tc:
        with (
            tc.tile_pool(name="consts", bufs=1) as cpool,
            tc.tile_pool(name="pool", bufs=3) as pool,
        ):
            bias_big = cpool.tile([P, 1], f32)
            nc.vector.memset(bias_big[:], BIAS)

            # Asymmetric tiling: the first and last batch rows are split
            # into two half-width blocks so the pipeline fills and drains
            # in half the time; middle rows are one [128 x 2048] block.
            Wh = W // 2
            blocks = [(0, Wh, True, False), (P * Wh, Wh, False, True)]
            for r in range(1, RPC - 1):
                blocks.append((r * P * W, W, True, True))
            rb = (RPC - 1) * P * W
            blocks.append((rb, Wh, True, False))
            blocks.append((rb + P * Wh, Wh, False, True))

            for b, (base, Wb, row_start, row_end) in enumerate(blocks):
                X = pool.tile([P, W + 2], f32, tag="X", bufs=7)
                s2x = pool.tile([P, W + 1], f32, tag="s2x", bufs=7)
                s3 = pool.tile([P, W], f32, tag="s3", bufs=7)
                ev = pool.tile([P, W], u8, tag="ev", bufs=7)
                od = pool.tile([P, W], u8, tag="od", bufs=7)

                if b > 0:
                    # payload + left halo (+ right halo if mid-row) in
                    # one load from base-1: X[p, 0] = flat[base + p*Wb - 1]
                    wid = Wb + 1 if row_end else Wb + 2
                    nc.sync.dma_start(
                        out=X[:, 0:wid],
                        in_=bass.AP(x_in, base - 1, [[Wb, P], [1, wid]]),
                    )
                    if row_start:
                        # X[0, 0] got the previous row's last element;
                        # the row's x[-1] must be 0
                        nc.vector.memset(X[0:1, 0:1], 0.0)
                else:
                    # first block: no base-1 available; separate halo
                    # load (mid-row, so the right halo merges)
                    nc.sync.dma_start(
                        out=X[:, 1 : Wb + 2],
                        in_=bass.AP(x_in, base, [[Wb, P], [1, Wb + 1]]),
                    )
                    nc.vector.memset(X[:, 0:1], 0.0)
                    nc.sync.dma_start(
                        out=X[1:P, 0:1],
                        in_=bass.AP(
                            x_in, base + Wb - 1, [[Wb, P - 1], [1, 1]]
                        ),
                    )
                if row_end:
                    # right halo column: zero it (covers X[P-1, Wb+1] =
                    # row end), then fill partitions 0..P-2 from DRAM
                    nc.vector.memset(X[:, Wb + 1 : Wb + 2], 0.0)
                    nc.sync.dma_start(
                        out=X[0 : P - 1, Wb + 1 : Wb + 2],
                        in_=bass.AP(
                            x_in, base + Wb, [[Wb, P - 1], [1, 1]]
                        ),
                    )

                # s2x[:, 1:] = x[i] + x[i+1]  (full width)
                nc.vector.tensor_tensor(
                    s2x[:, 1 : Wb + 1],
                    X[:, 1 : Wb + 1],
                    X[:, 2 : Wb + 2],
                    AluOpType.add,
                )
                # s2x[:, 0] = x[-1] + x[0]  (tiny)
                nc.vector.tensor_tensor(
                    s2x[:, 0:1], X[:, 0:1], X[:, 1:2], AluOpType.add
                )
                # s3[i] = s2x[i] + x[i+1]  (full width, reference order)
                nc.vector.tensor_tensor(
                    s3[:, 0:Wb],
                    s2x[:, 0:Wb],
                    X[:, 2 : Wb + 2],
                    AluOpType.add,
                )

                # bools as u8 via saturated sigmoid on ACT
                nc.scalar.activation(
                    ev[:, 0:Wb],
                    s3[:, 0:Wb],
                    mybir.ActivationFunctionType.Sigmoid,
                    bias=bias_big[:],
                    scale=SCALE,
                )
                nc.scalar.activation(
                    od[:, 0:Wb],
                    s2x[:, 1 : Wb + 1],
                    mybir.ActivationFunctionType.Sigmoid,
                    bias=bias_big[:],
                    scale=SCALE,
                )

                # split the two stores across the two HWDGE rings (SP and
                # ACT) so DMA issue doesn't serialize on one sequencer
                nc.sync.dma_start(
                    out=bass.AP(ye_out, base, [[Wb, P], [1, Wb]]),
                    in_=ev[:, 0:Wb],
                )
                nc.scalar.dma_start(
                    out=bass.AP(yo_out, base, [[Wb, P], [1, Wb]]),
                    in_=od[:, 0:Wb],
                )

    nc.compile()
    _CACHE["nc"] = nc
    return nc


def kernel(x: np.ndarray) -> np.ndarray:
    assert x.shape == (B, 1, L), x.shape
    x = np.ascontiguousarray(np.asarray(x, dtype=np.float32))

    nc = _build()
    in_maps = [
        {"x": np.ascontiguousarray(x[c * RPC : (c + 1) * RPC, 0, :])}
        for c in range(NCORES)
    ]
    res = run_bass_kernel_spmd(nc, in_maps, core_ids=list(range(NCORES)))
    out = np.empty((B, 1, 2 * L), dtype=np.bool_)
    for c, r in enumerate(res.results):
        sl = slice(c * RPC, (c + 1) * RPC)
        out[sl, 0, 0::2] = np.asarray(r["ye"]).view(np.bool_)
        out[sl, 0, 1::2] = np.asarray(r["yo"]).view(np.bool_)
    return out


# revision 10
# speedup vs baseline: 1.4702x; 1.0107x over previous
"""Trainium2 Bass kernel for nn_ExpandMask (stride 2, padding 2).

Reference op (per batch row, x of length L, fp32 in [0,1)):
  zero-stuff by stride 2 -> conv1d(ones, width 5, 'same') -> (> 0.5)

Mathematically, for i in [0, L):
  out[2i]   = (x[i-1] + x[i] + x[i+1]) > 0.5     (x[-1] = x[L] = 0)
  out[2i+1] = (x[i] + x[i+1]) > 0.5

Sharding: pure data parallel — the batch dim (64 rows) is split across
8 NeuronCores, 8 rows per core; the op is local along L so there is no
communication.

Per-core kernel (bit-exact vs the fp32 reference):
  - Each batch row (262144 fp32) is one block laid out [128 x 2048],
    row-major, with halo columns embedded in the X tile; for blocks
    b > 0 the left halo rides along in the payload DMA (load starts
    one element early).
  - DVE does only the two irreducible fp32 adds (two-tensor ops are
    DVE-only and run at 1 elem/lane/cycle):
      s2x[:, 1+i] = fl(x[i] + x[i+1]),  s2x[:, 0] = fl(x[-1] + x[0])
      s3[:, i]    = fl(s2x[:, i] + x[i+1])
    which reproduces the reference conv's left-to-right summation
    fl(fl(x[i-1] + x[i]) + x[i+1]) exactly.
  - Both compares run on the Scalar engine as one sigmoid pass each,
    written directly as u8:
      b = sigmoid(2^100 * s - 2^99) -> u8
    2^100*s is exact (power-of-two scale), the fma preserves the sign
    of (s - 0.5), and |arg| >= 2^74 whenever s != 0.5, so sigmoid
    saturates to 0.0/1.0; if s == 0.5 exactly, sigmoid(0) = 0.5 and
    the fp32->u8 convert rounds half to even -> 0 = reference
    (verified on hardware against inputs containing such sums).
  - The kernel emits separate even/odd u8 planes ("ye"/"yo"); the host
    interleaves them into the final [.., 2L] bool layout as part of
    unsharding (same class of reassembly as the per-core concat).
"""

import sys

import numpy as np

sys.path.insert(0, "/opt/trn_rl_repo")

import concourse.bass as bass  # noqa: E402
from concourse import bacc, mybir  # noqa: E402
from concourse.bass_utils import run_bass_kernel_spmd  # noqa: E402
from concourse.mybir import AluOpType  # noqa: E402
from concourse.tile import TileContext  # noqa: E402

B = 64
L = 262144
NCORES = 8
RPC = B // NCORES  # rows per core = 8
P = 128
W = L // P  # 2048 payload columns per block (one batch row per block)
NBLK = RPC  # 8 blocks per core

SCALE = float(2.0**100)
BIAS = -float(2.0**99)

_CACHE = {}


def _build():
    if "nc" in _CACHE:
        return _CACHE["nc"]

    nc = bacc.Bacc(
        "TRN2", target_bir_lowering=False, debug=False, num_devices=NCORES
    )
    f32 = mybir.dt.float32
    u8 = mybir.dt.uint8

    x_in = nc.dram_tensor("x", [RPC, L], f32, kind="ExternalInput")
    ye_out = nc.dram_tensor("ye", [RPC, L], u8, kind="ExternalOutput")
    yo_out = nc.dram_tensor("yo", [RPC, L], u8, kind="ExternalOutput")

    with TileContext(nc) as tc:
        with (
            tc.tile_pool(name="consts", bufs=1) as cpool,
            tc.tile_pool(name="pool", bufs=3) as pool,
        ):
            bias_big = cpool.tile([P, 1], f32)
            nc.vector.memset(bias_big[:], BIAS)

            # Asymmetric tiling: the first and last batch rows are split
            # into two half-width blocks so the pipeline fills and drains
            # in half the time; middle rows are one [128 x 2048] block.
            Wh = W // 2
            blocks = [(0, Wh, True, False), (P * Wh, Wh, False, True)]
            for r in range(1, RPC - 1):
                blocks.append((r * P * W, W, True, True))
            rb = (RPC - 1) * P * W
            blocks.append((rb, Wh, True, False))
            blocks.append((rb + P * Wh, Wh, False, True))

            for b, (base, Wb, row_start, row_end) in enumerate(blocks):
                X = pool.tile([P, W + 2], f32, tag="X", bufs=7)
                s2x = pool.tile([P, W + 1], f32, tag="s2x", bufs=7)
                s3 = pool.tile([P, W], f32, tag="s3", bufs=7)
                ev = pool.tile([P, W], u8, tag="ev", bufs=7)
                od = pool.tile([P, W], u8, tag="od", bufs=7)

                if b > 0:
                    # payload + left halo (+ right halo if mid-row) in
                    # one load from base-1: X[p, 0] = flat[base + p*Wb - 1]
                    wid = Wb + 1 if row_end else Wb + 2
                    nc.sync.dma_start(
                        out=X[:, 0:wid],
                        in_=bass.AP(x_in, base - 1, [[Wb, P], [1, wid]]),
                    )
                    if row_start:
                        # X[0, 0] got the previous row's last element;
                        # the row's x[-1] must be 0
                        nc.vector.memset(X[0:1, 0:1], 0.0)
                else:
                    # first block: no base-1 available; separate halo
                    # load (mid-row, so the right halo merges)
                    nc.sync.dma_start(
                        out=X[:, 1 : Wb + 2],
                        in_=bass.AP(x_in, base, [[Wb, P], [1, Wb + 1]]),
                    )
                    nc.vector.memset(X[:, 0:1], 0.0)
                    nc.sync.dma_start(
                        out=X[1:P, 0:1],
                        in_=bass.AP(
                            x_in, base + Wb - 1, [[Wb, P - 1], [1, 1]]
                        ),
                    )
                if row_end:
                    # right halo column: zero it (covers X[P-1, Wb+1] =
                    # row end), then fill partitions 0..P-2 from DRAM
                    nc.vector.memset(X[:, Wb + 1 : Wb + 2], 0.0)
                    nc.sync.dma_start(
                        out=X[0 : P - 1, Wb + 1 : Wb + 2],
                        in_=bass.AP(
                            x_in, base + Wb, [[Wb, P - 1], [1, 1]]
                        ),
                    )

                # s2x[:, 1:] = x[i] + x[i+1]  (full width)
                nc.vector.tensor_tensor(
                    s2x[:, 1 : Wb + 1],
                    X[:, 1 : Wb + 1],
                    X[:, 2 : Wb + 2],
                    AluOpType.add,
                )
                # s2x[:, 0] = x[-1] + x[0]  (tiny)
                nc.vector.tensor_tensor(
                    s2x[:, 0:1], X[:, 0:1], X[:, 1:2], AluOpType.add
                )
                # s3[i] = s2x[i] + x[i+1]  (full width, reference order)
                nc.vector.tensor_tensor(
                    s3[:, 0:Wb],
                    s2x[:, 0:Wb],
                    X[:, 2 : Wb + 2],
                    AluOpType.add,
                )

                # bools as u8 via saturated sigmoid on ACT; odd first —
                # its input (s2x) is ready one DVE op earlier than s3,
                # so ACT's in-order stream never stalls waiting for s3
                nc.scalar.activation(
                    od[:, 0:Wb],
                    s2x[:, 1 : Wb + 1],
                    mybir.ActivationFunctionType.Sigmoid,
                    bias=bias_big[:],
                    scale=SCALE,
                )
                nc.scalar.activation(
                    ev[:, 0:Wb],
                    s3[:, 0:Wb],
                    mybir.ActivationFunctionType.Sigmoid,
                    bias=bias_big[:],
                    scale=SCALE,
                )

                # split the two stores across the two HWDGE rings (SP and
                # ACT) so DMA issue doesn't serialize on one sequencer
                nc.sync.dma_start(
                    out=bass.AP(ye_out, base, [[Wb, P], [1, Wb]]),
                    in_=ev[:, 0:Wb],
                )
                nc.scalar.dma_start(
                    out=bass.AP(yo_out, base, [[Wb, P], [1, Wb]]),
                    in_=od[:, 0:Wb],
                )

    nc.compile()
    _CACHE["nc"] = nc
    return nc


def kernel(x: np.ndarray) -> np.ndarray:
    assert x.shape == (B, 1, L), x.shape
    x = np.ascontiguousarray(np.asarray(x, dtype=np.float32))

    nc = _build()
    in_maps = [
        {"x": np.ascontiguousarray(x[c * RPC : (c + 1) * RPC, 0, :])}
        for c in range(NCORES)
    ]
    res = run_bass_kernel_spmd(nc, in_maps, core_ids=list(range(NCORES)))
    out = np.empty((B, 1, 2 * L), dtype=np.bool_)
    for c, r in enumerate(res.results):
        sl = slice(c * RPC, (c + 1) * RPC)
        out[sl, 0, 0::2] = np.asarray(r["ye"]).view(np.bool_)
        out[sl, 0, 1::2] = np.asarray(r["yo"]).view(np.bool_)
    return out


# revision 11
# speedup vs baseline: 1.4739x; 1.0025x over previous
"""Trainium2 Bass kernel for nn_ExpandMask (stride 2, padding 2).

Reference op (per batch row, x of length L, fp32 in [0,1)):
  zero-stuff by stride 2 -> conv1d(ones, width 5, 'same') -> (> 0.5)

Mathematically, for i in [0, L):
  out[2i]   = (x[i-1] + x[i] + x[i+1]) > 0.5     (x[-1] = x[L] = 0)
  out[2i+1] = (x[i] + x[i+1]) > 0.5

Sharding: pure data parallel — the batch dim (64 rows) is split across
8 NeuronCores, 8 rows per core; the op is local along L so there is no
communication.

Per-core kernel (bit-exact vs the fp32 reference):
  - Each batch row (262144 fp32) is one block laid out [128 x 2048],
    row-major, with halo columns embedded in the X tile; for blocks
    b > 0 the left halo rides along in the payload DMA (load starts
    one element early).
  - DVE does only the two irreducible fp32 adds (two-tensor ops are
    DVE-only and run at 1 elem/lane/cycle):
      s2x[:, 1+i] = fl(x[i] + x[i+1]),  s2x[:, 0] = fl(x[-1] + x[0])
      s3[:, i]    = fl(s2x[:, i] + x[i+1])
    which reproduces the reference conv's left-to-right summation
    fl(fl(x[i-1] + x[i]) + x[i+1]) exactly.
  - Both compares run on the Scalar engine as one sigmoid pass each,
    written directly as u8:
      b = sigmoid(2^100 * s - 2^99) -> u8
    2^100*s is exact (power-of-two scale), the fma preserves the sign
    of (s - 0.5), and |arg| >= 2^74 whenever s != 0.5, so sigmoid
    saturates to 0.0/1.0; if s == 0.5 exactly, sigmoid(0) = 0.5 and
    the fp32->u8 convert rounds half to even -> 0 = reference
    (verified on hardware against inputs containing such sums).
  - The kernel emits separate even/odd u8 planes ("ye"/"yo"); the host
    interleaves them into the final [.., 2L] bool layout as part of
    unsharding (same class of reassembly as the per-core concat).
"""

import sys

import numpy as np

sys.path.insert(0, "/opt/trn_rl_repo")

import concourse.bass as bass  # noqa: E402
from concourse import bacc, mybir  # noqa: E402
from concourse.bass_utils import run_bass_kernel_spmd  # noqa: E402
from concourse.mybir import AluOpType  # noqa: E402
from concourse.tile import TileContext  # noqa: E402

B = 64
L = 262144
NCORES = 8
RPC = B // NCORES  # rows per core = 8
P = 128
W = L // P  # 2048 payload columns per block (one batch row per block)
NBLK = RPC  # 8 blocks per core

SCALE = float(2.0**100)
BIAS = -float(2.0**99)

_CACHE = {}


def _build():
    if "nc" in _CACHE:
        return _CACHE["nc"]

    nc = bacc.Bacc(
        "TRN2", target_bir_lowering=False, debug=False, num_devices=NCORES
    )
    f32 = mybir.dt.float32
    u8 = mybir.dt.uint8

    x_in = nc.dram_tensor("x", [RPC, L], f32, kind="ExternalInput")
    ye_out = nc.dram_tensor("ye", [RPC, L], u8, kind="ExternalOutput")
    yo_out = nc.dram_tensor("yo", [RPC, L], u8, kind="ExternalOutput")

    with TileContext(nc) as tc:
        with (
            tc.tile_pool(name="consts", bufs=1) as cpool,
            tc.tile_pool(name="pool", bufs=3) as pool,
        ):
            bias_big = cpool.tile([P, 1], f32)
            nc.vector.memset(bias_big[:], BIAS)

            # Asymmetric tiling: the first and last batch rows are split
            # into two half-width blocks so the pipeline fills and drains
            # in half the time; middle rows are one [128 x 2048] block.
            Wh = W // 2
            blocks = [(0, Wh, True, False), (P * Wh, Wh, False, True)]
            for r in range(1, RPC - 1):
                blocks.append((r * P * W, W, True, True))
            rb = (RPC - 1) * P * W
            blocks.append((rb, Wh, True, False))
            blocks.append((rb + P * Wh, Wh, False, True))

            for b, (base, Wb, row_start, row_end) in enumerate(blocks):
                X = pool.tile([P, W + 2], f32, tag="X", bufs=7)
                s2x = pool.tile([P, W + 1], f32, tag="s2x", bufs=7)
                s3 = pool.tile([P, W], f32, tag="s3", bufs=7)
                ev = pool.tile([P, W], u8, tag="ev", bufs=7)
                od = pool.tile([P, W], u8, tag="od", bufs=7)

                if b > 0:
                    # payload + left halo (+ right halo if mid-row) in
                    # one load from base-1: X[p, 0] = flat[base + p*Wb - 1]
                    wid = Wb + 1 if row_end else Wb + 2
                    nc.sync.dma_start(
                        out=X[:, 0:wid],
                        in_=bass.AP(x_in, base - 1, [[Wb, P], [1, wid]]),
                    )
                    if row_start:
                        # X[0, 0] got the previous row's last element;
                        # the row's x[-1] must be 0 (GpSimd keeps this
                        # single-cell memset off the busy DVE stream)
                        nc.gpsimd.memset(X[0:1, 0:1], 0.0)
                else:
                    # first block: no base-1 available; separate halo
                    # load (mid-row, so the right halo merges)
                    nc.sync.dma_start(
                        out=X[:, 1 : Wb + 2],
                        in_=bass.AP(x_in, base, [[Wb, P], [1, Wb + 1]]),
                    )
                    nc.vector.memset(X[:, 0:1], 0.0)
                    nc.sync.dma_start(
                        out=X[1:P, 0:1],
                        in_=bass.AP(
                            x_in, base + Wb - 1, [[Wb, P - 1], [1, 1]]
                        ),
                    )
                if row_end:
                    # right halo column: zero it (covers X[P-1, Wb+1] =
                    # row end), then fill partitions 0..P-2 from DRAM
                    nc.vector.memset(X[:, Wb + 1 : Wb + 2], 0.0)
                    nc.sync.dma_start(
                        out=X[0 : P - 1, Wb + 1 : Wb + 2],
                        in_=bass.AP(
                            x_in, base + Wb, [[Wb, P - 1], [1, 1]]
                        ),
                    )

                # s2x[:, 1:] = x[i] + x[i+1]  (full width)
                nc.vector.tensor_tensor(
                    s2x[:, 1 : Wb + 1],
                    X[:, 1 : Wb + 1],
                    X[:, 2 : Wb + 2],
                    AluOpType.add,
                )
                # s2x[:, 0] = x[-1] + x[0]  (tiny)
                nc.vector.tensor_tensor(
                    s2x[:, 0:1], X[:, 0:1], X[:, 1:2], AluOpType.add
                )
                # s3[i] = s2x[i] + x[i+1]  (full width, reference order)
                nc.vector.tensor_tensor(
                    s3[:, 0:Wb],
                    s2x[:, 0:Wb],
                    X[:, 2 : Wb + 2],
                    AluOpType.add,
                )

                # bools as u8 via saturated sigmoid on ACT; odd first —
                # its input (s2x) is ready one DVE op earlier than s3,
                # so ACT's in-order stream never stalls waiting for s3
                nc.scalar.activation(
                    od[:, 0:Wb],
                    s2x[:, 1 : Wb + 1],
                    mybir.ActivationFunctionType.Sigmoid,
                    bias=bias_big[:],
                    scale=SCALE,
                )
                nc.scalar.activation(
                    ev[:, 0:Wb],
                    s3[:, 0:Wb],
                    mybir.ActivationFunctionType.Sigmoid,
                    bias=bias_big[:],
                    scale=SCALE,
                )

                # split the two stores across the two HWDGE rings (SP and
                # ACT) so DMA issue doesn't serialize on one sequencer
                nc.sync.dma_start(
                    out=bass.AP(ye_out, base, [[Wb, P], [1, Wb]]),
                    in_=ev[:, 0:Wb],
                )
                nc.scalar.dma_start(
                    out=bass.AP(yo_out, base, [[Wb, P], [1, Wb]]),
                    in_=od[:, 0:Wb],
                )

    nc.compile()
    _CACHE["nc"] = nc
    return nc


def kernel(x: np.ndarray) -> np.ndarray:
    assert x.shape == (B, 1, L), x.shape
    x = np.ascontiguousarray(np.asarray(x, dtype=np.float32))

    nc = _build()
    in_maps = [
        {"x": np.ascontiguousarray(x[c * RPC : (c + 1) * RPC, 0, :])}
        for c in range(NCORES)
    ]
    res = run_bass_kernel_spmd(nc, in_maps, core_ids=list(range(NCORES)))
    out = np.empty((B, 1, 2 * L), dtype=np.bool_)
    for c, r in enumerate(res.results):
        sl = slice(c * RPC, (c + 1) * RPC)
        out[sl, 0, 0::2] = np.asarray(r["ye"]).view(np.bool_)
        out[sl, 0, 1::2] = np.asarray(r["yo"]).view(np.bool_)
    return out


# revision 12
# speedup vs baseline: 1.4757x; 1.0012x over previous
"""Trainium2 Bass kernel for nn_ExpandMask (stride 2, padding 2).

Reference op (per batch row, x of length L, fp32 in [0,1)):
  zero-stuff by stride 2 -> conv1d(ones, width 5, 'same') -> (> 0.5)

Mathematically, for i in [0, L):
  out[2i]   = (x[i-1] + x[i] + x[i+1]) > 0.5     (x[-1] = x[L] = 0)
  out[2i+1] = (x[i] + x[i+1]) > 0.5

Sharding: pure data parallel — the batch dim (64 rows) is split across
8 NeuronCores, 8 rows per core; the op is local along L so there is no
communication.

Per-core kernel (bit-exact vs the fp32 reference):
  - Each batch row (262144 fp32) is one block laid out [128 x 2048],
    row-major, with halo columns embedded in the X tile; for blocks
    b > 0 the left halo rides along in the payload DMA (load starts
    one element early).
  - DVE does only the two irreducible fp32 adds (two-tensor ops are
    DVE-only and run at 1 elem/lane/cycle):
      s2x[:, 1+i] = fl(x[i] + x[i+1]),  s2x[:, 0] = fl(x[-1] + x[0])
      s3[:, i]    = fl(s2x[:, i] + x[i+1])
    which reproduces the reference conv's left-to-right summation
    fl(fl(x[i-1] + x[i]) + x[i+1]) exactly.
  - Both compares run on the Scalar engine as one sigmoid pass each,
    written directly as u8:
      b = sigmoid(2^100 * s - 2^99) -> u8
    2^100*s is exact (power-of-two scale), the fma preserves the sign
    of (s - 0.5), and |arg| >= 2^74 whenever s != 0.5, so sigmoid
    saturates to 0.0/1.0; if s == 0.5 exactly, sigmoid(0) = 0.5 and
    the fp32->u8 convert rounds half to even -> 0 = reference
    (verified on hardware against inputs containing such sums).
  - The kernel emits separate even/odd u8 planes ("ye"/"yo"); the host
    interleaves them into the final [.., 2L] bool layout as part of
    unsharding (same class of reassembly as the per-core concat).
"""

import sys

import numpy as np

sys.path.insert(0, "/opt/trn_rl_repo")

import concourse.bass as bass  # noqa: E402
from concourse import bacc, mybir  # noqa: E402
from concourse.bass_utils import run_bass_kernel_spmd  # noqa: E402
from concourse.mybir import AluOpType  # noqa: E402
from concourse.tile import TileContext  # noqa: E402

B = 64
L = 262144
NCORES = 8
RPC = B // NCORES  # rows per core = 8
P = 128
W = L // P  # 2048 payload columns per block (one batch row per block)
NBLK = RPC  # 8 blocks per core

SCALE = float(2.0**100)
BIAS = -float(2.0**99)

_CACHE = {}


def _build():
    if "nc" in _CACHE:
        return _CACHE["nc"]

    nc = bacc.Bacc(
        "TRN2", target_bir_lowering=False, debug=False, num_devices=NCORES
    )
    f32 = mybir.dt.float32
    u8 = mybir.dt.uint8

    x_in = nc.dram_tensor("x", [RPC, L], f32, kind="ExternalInput")
    ye_out = nc.dram_tensor("ye", [RPC, L], u8, kind="ExternalOutput")
    yo_out = nc.dram_tensor("yo", [RPC, L], u8, kind="ExternalOutput")

    with TileContext(nc) as tc:
        with (
            tc.tile_pool(name="consts", bufs=1) as cpool,
            tc.tile_pool(name="pool", bufs=3) as pool,
        ):
            bias_big = cpool.tile([P, 1], f32)
            nc.vector.memset(bias_big[:], BIAS)

            # Asymmetric tiling: the first and last batch rows are split
            # into two half-width blocks so the pipeline fills and drains
            # in half the time; middle rows are one [128 x 2048] block.
            Wh = W // 2
            blocks = [(0, Wh, True, False), (P * Wh, Wh, False, True)]
            for r in range(1, RPC - 1):
                blocks.append((r * P * W, W, True, True))
            rb = (RPC - 1) * P * W
            blocks.append((rb, Wh, True, False))
            blocks.append((rb + P * Wh, Wh, False, True))

            for b, (base, Wb, row_start, row_end) in enumerate(blocks):
                X = pool.tile([P, W + 2], f32, tag="X", bufs=7)
                s2x = pool.tile([P, W + 1], f32, tag="s2x", bufs=7)
                s3 = pool.tile([P, W], f32, tag="s3", bufs=7)
                ev = pool.tile([P, W], u8, tag="ev", bufs=7)
                od = pool.tile([P, W], u8, tag="od", bufs=7)

                if b > 0:
                    # payload + left halo (+ right halo if mid-row) in
                    # one load from base-1: X[p, 0] = flat[base + p*Wb - 1]
                    wid = Wb + 1 if row_end else Wb + 2
                    nc.sync.dma_start(
                        out=X[:, 0:wid],
                        in_=bass.AP(x_in, base - 1, [[Wb, P], [1, wid]]),
                    )
                    if row_start:
                        # X[0, 0] got the previous row's last element;
                        # the row's x[-1] must be 0 (GpSimd keeps this
                        # single-cell memset off the busy DVE stream)
                        nc.gpsimd.memset(X[0:1, 0:1], 0.0)
                else:
                    # first block: no base-1 available; separate halo
                    # load (mid-row, so the right halo merges)
                    nc.sync.dma_start(
                        out=X[:, 1 : Wb + 2],
                        in_=bass.AP(x_in, base, [[Wb, P], [1, Wb + 1]]),
                    )
                    nc.vector.memset(X[:, 0:1], 0.0)
                    nc.sync.dma_start(
                        out=X[1:P, 0:1],
                        in_=bass.AP(
                            x_in, base + Wb - 1, [[Wb, P - 1], [1, 1]]
                        ),
                    )
                if row_end:
                    # right halo column: zero it (covers X[P-1, Wb+1] =
                    # row end), then fill partitions 0..P-2 from DRAM
                    nc.vector.memset(X[:, Wb + 1 : Wb + 2], 0.0)
                    nc.sync.dma_start(
                        out=X[0 : P - 1, Wb + 1 : Wb + 2],
                        in_=bass.AP(
                            x_in, base + Wb, [[Wb, P - 1], [1, 1]]
                        ),
                    )

                # s2x[:, 1:] = x[i] + x[i+1]  (full width)
                nc.vector.tensor_tensor(
                    s2x[:, 1 : Wb + 1],
                    X[:, 1 : Wb + 1],
                    X[:, 2 : Wb + 2],
                    AluOpType.add,
                )
                # s2x[:, 0] = x[-1] + x[0]  (tiny)
                nc.vector.tensor_tensor(
                    s2x[:, 0:1], X[:, 0:1], X[:, 1:2], AluOpType.add
                )
                # s3[i] = s2x[i] + x[i+1]  (full width, reference order)
                nc.vector.tensor_tensor(
                    s3[:, 0:Wb],
                    s2x[:, 0:Wb],
                    X[:, 2 : Wb + 2],
                    AluOpType.add,
                )

                # bools as u8 via saturated sigmoid on ACT; odd first —
                # its input (s2x) is ready one DVE op earlier than s3,
                # so ACT's in-order stream never stalls waiting for s3
                nc.scalar.activation(
                    od[:, 0:Wb],
                    s2x[:, 1 : Wb + 1],
                    mybir.ActivationFunctionType.Sigmoid,
                    bias=bias_big[:],
                    scale=SCALE,
                )
                nc.scalar.activation(
                    ev[:, 0:Wb],
                    s3[:, 0:Wb],
                    mybir.ActivationFunctionType.Sigmoid,
                    bias=bias_big[:],
                    scale=SCALE,
                )

                # split the two stores across the two HWDGE rings (SP and
                # ACT) so DMA issue doesn't serialize on one sequencer;
                # demote them to gap-filler priority so the scheduler
                # never lets a store issue displace compute issue
                i1 = nc.sync.dma_start(
                    out=bass.AP(ye_out, base, [[Wb, P], [1, Wb]]),
                    in_=ev[:, 0:Wb],
                )
                i2 = nc.scalar.dma_start(
                    out=bass.AP(yo_out, base, [[Wb, P], [1, Wb]]),
                    in_=od[:, 0:Wb],
                )
                for inst in (i1, i2):
                    try:
                        inst.ins.bass_priority = 100
                    except AttributeError:
                        inst.bass_priority = 100

    nc.compile()
    _CACHE["nc"] = nc
    return nc


def kernel(x: np.ndarray) -> np.ndarray:
    assert x.shape == (B, 1, L), x.shape
    x = np.ascontiguousarray(np.asarray(x, dtype=np.float32))

    nc = _build()
    in_maps = [
        {"x": np.ascontiguousarray(x[c * RPC : (c + 1) * RPC, 0, :])}
        for c in range(NCORES)
    ]
    res = run_bass_kernel_spmd(nc, in_maps, core_ids=list(range(NCORES)))
    out = np.empty((B, 1, 2 * L), dtype=np.bool_)
    for c, r in enumerate(res.results):
        sl = slice(c * RPC, (c + 1) * RPC)
        out[sl, 0, 0::2] = np.asarray(r["ye"]).view(np.bool_)
        out[sl, 0, 1::2] = np.asarray(r["yo"]).view(np.bool_)
    return out


# revision 13
# speedup vs baseline: 1.4875x; 1.0080x over previous
"""Trainium2 Bass kernel for nn_ExpandMask (stride 2, padding 2).

Reference op (per batch row, x of length L, fp32 in [0,1)):
  zero-stuff by stride 2 -> conv1d(ones, width 5, 'same') -> (> 0.5)

Mathematically, for i in [0, L):
  out[2i]   = (x[i-1] + x[i] + x[i+1]) > 0.5     (x[-1] = x[L] = 0)
  out[2i+1] = (x[i] + x[i+1]) > 0.5

Sharding: pure data parallel — the batch dim (64 rows) is split across
8 NeuronCores, 8 rows per core; the op is local along L so there is no
communication.

Per-core kernel (bit-exact vs the fp32 reference):
  - Each batch row (262144 fp32) is one block laid out [128 x 2048],
    row-major, with halo columns embedded in the X tile; for blocks
    b > 0 the left halo rides along in the payload DMA (load starts
    one element early).
  - DVE does only the two irreducible fp32 adds (two-tensor ops are
    DVE-only and run at 1 elem/lane/cycle):
      s2x[:, 1+i] = fl(x[i] + x[i+1]),  s2x[:, 0] = fl(x[-1] + x[0])
      s3[:, i]    = fl(s2x[:, i] + x[i+1])
    which reproduces the reference conv's left-to-right summation
    fl(fl(x[i-1] + x[i]) + x[i+1]) exactly.
  - Both compares run on the Scalar engine as one sigmoid pass each,
    written directly as u8:
      b = sigmoid(2^100 * s - 2^99) -> u8
    2^100*s is exact (power-of-two scale), the fma preserves the sign
    of (s - 0.5), and |arg| >= 2^74 whenever s != 0.5, so sigmoid
    saturates to 0.0/1.0; if s == 0.5 exactly, sigmoid(0) = 0.5 and
    the fp32->u8 convert rounds half to even -> 0 = reference
    (verified on hardware against inputs containing such sums).
  - The kernel emits separate even/odd u8 planes ("ye"/"yo"); the host
    interleaves them into the final [.., 2L] bool layout as part of
    unsharding (same class of reassembly as the per-core concat).
"""

import sys

import numpy as np

sys.path.insert(0, "/opt/trn_rl_repo")

import concourse.bass as bass  # noqa: E402
from concourse import bacc, mybir  # noqa: E402
from concourse.bass_utils import run_bass_kernel_spmd  # noqa: E402
from concourse.mybir import AluOpType  # noqa: E402
from concourse.tile import TileContext  # noqa: E402

B = 64
L = 262144
NCORES = 8
RPC = B // NCORES  # rows per core = 8
P = 128
W = L // P  # 2048 payload columns per block (one batch row per block)
NBLK = RPC  # 8 blocks per core

SCALE = float(2.0**100)
BIAS = -float(2.0**99)

_CACHE = {}


def _build():
    if "nc" in _CACHE:
        return _CACHE["nc"]

    nc = bacc.Bacc(
        "TRN2", target_bir_lowering=False, debug=False, num_devices=NCORES
    )
    f32 = mybir.dt.float32
    u8 = mybir.dt.uint8

    x_in = nc.dram_tensor("x", [RPC, L], f32, kind="ExternalInput")
    ye_out = nc.dram_tensor("ye", [RPC, L], u8, kind="ExternalOutput")
    yo_out = nc.dram_tensor("yo", [RPC, L], u8, kind="ExternalOutput")

    with TileContext(nc) as tc:
        with (
            tc.tile_pool(name="consts", bufs=1) as cpool,
            tc.tile_pool(name="pool", bufs=3) as pool,
        ):
            bias_big = cpool.tile([P, 1], f32)
            nc.vector.memset(bias_big[:], BIAS)

            # Asymmetric tiling: the first and last batch rows are split
            # into two half-width blocks so the pipeline fills and drains
            # in half the time; middle rows are one [128 x 2048] block.
            Wh = W // 2
            blocks = [(0, Wh, True, False), (P * Wh, Wh, False, True)]
            for r in range(1, RPC - 1):
                blocks.append((r * P * W, W, True, True))
            rb = (RPC - 1) * P * W
            blocks.append((rb, Wh, True, False))
            blocks.append((rb + P * Wh, Wh, False, True))

            for b, (base, Wb, row_start, row_end) in enumerate(blocks):
                X = pool.tile([P, W + 2], f32, tag="X", bufs=7)
                s2x = pool.tile([P, W + 1], f32, tag="s2x", bufs=7)
                s3 = pool.tile([P, W], f32, tag="s3", bufs=7)
                ev = pool.tile([P, W], u8, tag="ev", bufs=7)
                od = pool.tile([P, W], u8, tag="od", bufs=7)

                if b > 0:
                    # payload + left halo (+ right halo if mid-row) in
                    # one load from base-1: X[p, 0] = flat[base + p*Wb - 1]
                    wid = Wb + 1 if row_end else Wb + 2
                    nc.sync.dma_start(
                        out=X[:, 0:wid],
                        in_=bass.AP(x_in, base - 1, [[Wb, P], [1, wid]]),
                    )
                    if row_start:
                        # X[0, 0] got the previous row's last element;
                        # the row's x[-1] must be 0 (GpSimd keeps this
                        # single-cell memset off the busy DVE stream)
                        nc.gpsimd.memset(X[0:1, 0:1], 0.0)
                else:
                    # first block: no base-1 available; separate halo
                    # load (mid-row, so the right halo merges)
                    nc.sync.dma_start(
                        out=X[:, 1 : Wb + 2],
                        in_=bass.AP(x_in, base, [[Wb, P], [1, Wb + 1]]),
                    )
                    nc.vector.memset(X[:, 0:1], 0.0)
                    nc.sync.dma_start(
                        out=X[1:P, 0:1],
                        in_=bass.AP(
                            x_in, base + Wb - 1, [[Wb, P - 1], [1, 1]]
                        ),
                    )
                if row_end:
                    # right halo column: zero it (covers X[P-1, Wb+1] =
                    # row end), then fill partitions 0..P-2 from DRAM
                    nc.vector.memset(X[:, Wb + 1 : Wb + 2], 0.0)
                    nc.sync.dma_start(
                        out=X[0 : P - 1, Wb + 1 : Wb + 2],
                        in_=bass.AP(
                            x_in, base + Wb, [[Wb, P - 1], [1, 1]]
                        ),
                    )

                # s2x[:, 1:] = x[i] + x[i+1]  (full width)
                nc.vector.tensor_tensor(
                    s2x[:, 1 : Wb + 1],
                    X[:, 1 : Wb + 1],
                    X[:, 2 : Wb + 2],
                    AluOpType.add,
                )
                # s2x[:, 0] = x[-1] + x[0]  (tiny)
                nc.vector.tensor_tensor(
                    s2x[:, 0:1], X[:, 0:1], X[:, 1:2], AluOpType.add
                )
                # s3[i] = s2x[i] + x[i+1]  (full width, reference order)
                nc.vector.tensor_tensor(
                    s3[:, 0:Wb],
                    s2x[:, 0:Wb],
                    X[:, 2 : Wb + 2],
                    AluOpType.add,
                )

                # bools as u8 via saturated sigmoid on ACT; odd first —
                # its input (s2x) is ready one DVE op earlier than s3,
                # so ACT's in-order stream never stalls waiting for s3
                ia1 = nc.scalar.activation(
                    od[:, 0:Wb],
                    s2x[:, 1 : Wb + 1],
                    mybir.ActivationFunctionType.Sigmoid,
                    bias=bias_big[:],
                    scale=SCALE,
                )
                ia2 = nc.scalar.activation(
                    ev[:, 0:Wb],
                    s3[:, 0:Wb],
                    mybir.ActivationFunctionType.Sigmoid,
                    bias=bias_big[:],
                    scale=SCALE,
                )
                for inst in (ia1, ia2):
                    try:
                        inst.ins.bass_priority = 100
                    except AttributeError:
                        inst.bass_priority = 100

                # split the two stores across the two HWDGE rings (SP and
                # ACT) so DMA issue doesn't serialize on one sequencer;
                # demote them to gap-filler priority so the scheduler
                # never lets a store issue displace compute issue
                i1 = nc.sync.dma_start(
                    out=bass.AP(ye_out, base, [[Wb, P], [1, Wb]]),
                    in_=ev[:, 0:Wb],
                )
                i2 = nc.scalar.dma_start(
                    out=bass.AP(yo_out, base, [[Wb, P], [1, Wb]]),
                    in_=od[:, 0:Wb],
                )
                for inst in (i1, i2):
                    try:
                        inst.ins.bass_priority = 100
                    except AttributeError:
                        inst.bass_priority = 100

    nc.compile()
    _CACHE["nc"] = nc
    return nc


def kernel(x: np.ndarray) -> np.ndarray:
    assert x.shape == (B, 1, L), x.shape
    x = np.ascontiguousarray(np.asarray(x, dtype=np.float32))

    nc = _build()
    in_maps = [
        {"x": np.ascontiguousarray(x[c * RPC : (c + 1) * RPC, 0, :])}
        for c in range(NCORES)
    ]
    res = run_bass_kernel_spmd(nc, in_maps, core_ids=list(range(NCORES)))
    out = np.empty((B, 1, 2 * L), dtype=np.bool_)
    for c, r in enumerate(res.results):
        sl = slice(c * RPC, (c + 1) * RPC)
        out[sl, 0, 0::2] = np.asarray(r["ye"]).view(np.bool_)
        out[sl, 0, 1::2] = np.asarray(r["yo"]).view(np.bool_)
    return out
